# revision 1
# baseline (speedup 1.0000x reference)
"""Trainium2 Bass kernel for nn_Conv2DMod (StyleGAN2-style modulated 3x3 conv).

Problem: x[8,64,256,256], s[8,64], weight[64,64,3,3] (f32)
  w = weight * (s+1) per sample; demod by rsqrt(sum w^2 over (Cin,K,K));
  out[b] = conv2d(x[b], w_b, pad=1).

Sharding: data-parallel over batch. 8 samples -> 8 NeuronCores, one each.

Per-core algorithm (V2, bf16 crossed 4-cell):
  - weight prep on-chip in f32 (modulate by s+1, demodulate), transposed to
    lhsT layout [Cin, Cout] per kernel position, cast to bf16, replicated to
    both SBUF partition halves.
  - conv as shift-matmul over 9 kernel positions; x cast to bf16 on load
    (SWDGE cast DMA), rows processed as two concurrent 32-row blocks with
    1-row halos, columns padded to 258 so every shift is an AP offset.
  - PE runs as 4 independent 64x64 cells (row tiles = block0/block1 data,
    col tiles = psum partition halves). Per block, even kernel positions
    accumulate in one psum bank, odd in the other, crossed so each bank
    holds one block's partial per partition half:
       psumE[0:64] = block0 even | psumE[64:128] = block1 odd
       psumO[0:64] = block1 even | psumO[64:128] = block0 odd
  - evacuation per chunk-pair (2 rows x 2 blocks): ACT full-lane copy of
    psumE + 2 cross-base DVE adds of psumO halves; staged in SBUF
    (partition half = block) and DMA'd out on the HWDGE ring (x loads go
    via SWDGE, so loads and stores use different paths).
"""

import numpy as np

import concourse.bacc as bacc
import concourse.mybir as mybir
import concourse.tile as tile
from concourse.bass import ts
from concourse.bass_utils import run_bass_kernel_spmd
from concourse.masks import make_identity

F32 = mybir.dt.float32
BF16 = mybir.dt.bfloat16

B, CIN, COUT, KK, H, W = 8, 64, 64, 3, 256, 256
EPS = 1e-8
PW = W + 2          # padded row width
HB = 32             # output rows per block
NBI = H // (2 * HB)  # pair-iterations (4)
NCHUNK = HB // 2    # chunk-pairs per pair-iteration (16)
FLUSH = 8           # chunk-pairs per stage flush

EVEN = [0, 2, 4, 6, 8]
ODD = [1, 3, 5, 7]


def build_nc():
    nc = bacc.Bacc("TRN2")
    x = nc.dram_tensor("x", [CIN, H, W], F32, kind="ExternalInput")
    s = nc.dram_tensor("s", [1, CIN], F32, kind="ExternalInput")
    wgt = nc.dram_tensor("wgt", [COUT, CIN * 9], F32, kind="ExternalInput")
    out = nc.dram_tensor("out", [COUT, H, W], F32, kind="ExternalOutput")

    with tile.TileContext(nc) as tc:
        with tc.tile_pool(name="const", bufs=1) as constp:
            ident = constp.tile([64, 64], F32)
            make_identity(nc, ident)
            w2 = constp.tile([128, 9 * 64], BF16)

            # ---- weight prep (f32 math, bf16 result) ----
            with (
                tc.tile_pool(name="prep", bufs=1) as prepp,
                tc.tile_pool(name="prep_ps", bufs=2, space="PSUM") as prep_ps,
            ):
                w_o = prepp.tile([64, 64, 9], F32)     # [o, i, p]
                nc.sync.dma_start(out=w_o[:, :, :], in_=wgt[:, :])
                s_b = prepp.tile([64, 64], F32)        # [o, i] = s[i] bcast
                nc.gpsimd.dma_start(out=s_b[:, :], in_=s[0:1, :].to_broadcast((64, 64)))
                nc.vector.tensor_scalar_add(s_b[:, :], s_b[:, :], 1.0)

                wmod = prepp.tile([64, 64, 9], F32)
                nc.vector.tensor_mul(
                    wmod[:, :, :], w_o[:, :, :],
                    s_b[:, :].unsqueeze(2).to_broadcast((64, 64, 9)),
                )
                sq = prepp.tile([64, 64, 9], F32)
                nc.vector.tensor_mul(sq[:, :, :], wmod[:, :, :], wmod[:, :, :])
                ssum = prepp.tile([64, 1], F32)
                nc.vector.reduce_sum(out=ssum[:, :], in_=sq[:, :, :],
                                     axis=mybir.AxisListType.XY)
                epst = prepp.tile([64, 1], F32)
                nc.vector.memset(epst[:, :], EPS)
                dtmp = prepp.tile([64, 1], F32)
                nc.scalar.activation(dtmp[:, :], ssum[:, :],
                                     mybir.ActivationFunctionType.Sqrt,
                                     bias=epst[:, :])
                d_col = prepp.tile([64, 1], F32)
                nc.vector.reciprocal(d_col[:, :], dtmp[:, :])
                wfin = prepp.tile([64, 64, 9], F32)    # [o, i, p] final weights
                nc.vector.tensor_scalar_mul(wfin[:, :, :], wmod[:, :, :], d_col[:, :])

                # transpose each position [o,i] -> [i,o], write into w2 as bf16
                for p in range(9):
                    ps_t = prep_ps.tile([64, 64], F32, name=f"ps_t{p}", tag="ps_t")
                    nc.tensor.transpose(ps_t[:, :], wfin[:, :, p], ident[:, :])
                    nc.vector.tensor_copy(w2[0:64, ts(p, 64)], ps_t[:, :])
                # replicate to partitions 64-127
                nc.sync.dma_start(out=w2[64:128, :], in_=w2[0:64, :])

            # ---- main conv loop ----
            with (
                tc.tile_pool(name="xpool", bufs=2) as xpool,
                tc.tile_pool(name="stpool", bufs=2) as stpool,
                tc.tile_pool(name="pspool", bufs=2, space="PSUM") as pspool,
            ):
                for i in range(NBI):
                    xt = xpool.tile([128, HB + 2, PW], BF16, name=f"xt{i}", tag="xt")
                    # zero the column pads
                    nc.vector.memset(xt[:, :, 0:1], 0.0)
                    nc.vector.memset(xt[:, :, PW - 1:PW], 0.0)
                    # block0 rows [64i-1, 64i+33) -> partitions 0-63 (SWDGE cast)
                    lo = 64 * i - 1
                    if i == 0:
                        nc.vector.memset(xt[0:64, 0:1, :], 0.0)
                        # split so the first chunks' rows land fast
                        nc.gpsimd.dma_start(out=xt[0:64, 1:8, 1:W + 1],
                                            in_=x[:, 0:7, :])
                        nc.gpsimd.dma_start(out=xt[64:128, 0:8, 1:W + 1],
                                            in_=x[:, HB - 1:HB + 7, :])
                        nc.gpsimd.dma_start(out=xt[0:64, 8:HB + 2, 1:W + 1],
                                            in_=x[:, 7:HB + 1, :])
                        nc.gpsimd.dma_start(out=xt[64:128, 8:HB + 2, 1:W + 1],
                                            in_=x[:, HB + 7:2 * HB + 1, :])
                    else:
                        nc.gpsimd.dma_start(out=xt[0:64, :, 1:W + 1],
                                            in_=x[:, lo:lo + HB + 2, :])
                        # block1 rows [64i+31, 64i+65) -> partitions 64-127
                        hi = 64 * i + HB - 1
                        if i == NBI - 1:
                            nc.gpsimd.dma_start(out=xt[64:128, 0:HB + 1, 1:W + 1],
                                                in_=x[:, hi:H, :])
                            nc.vector.memset(xt[64:128, HB + 1:HB + 2, :], 0.0)
                        else:
                            nc.gpsimd.dma_start(out=xt[64:128, :, 1:W + 1],
                                                in_=x[:, hi:hi + HB + 2, :])

                    for half in range(NCHUNK // FLUSH):
                        stage = stpool.tile([128, FLUSH * 512], F32,
                                            name=f"stage{i}_{half}", tag="stage")
                        for jj in range(FLUSH):
                            j = half * FLUSH + jj
                            psE = pspool.tile([128, 512], F32,
                                              name=f"psE{i}_{j}", tag="psE")
                            psO = pspool.tile([128, 512], F32,
                                              name=f"psO{i}_{j}", tag="psO")
                            # cell -> (psum tile, partition half):
                            #  (b=0, even) -> psE[0:64]   (b=0, odd) -> psO[64:128]
                            #  (b=1, even) -> psO[0:64]   (b=1, odd) -> psE[64:128]
                            for r in range(5):
                                for par in range(2):       # 0=even, 1=odd
                                    if par == 1 and r >= len(ODD):
                                        continue
                                    p = (EVEN, ODD)[par][r]
                                    dy, dx = divmod(p, 3)
                                    for b in range(2):
                                        if b == 0 and par == 0:
                                            outap = psE[0:64, :]; tp = (0, 0)
                                        elif b == 0 and par == 1:
                                            outap = psO[64:128, :]; tp = (0, 64)
                                        elif b == 1 and par == 0:
                                            outap = psO[0:64, :]; tp = (64, 0)
                                        else:
                                            outap = psE[64:128, :]; tp = (64, 64)
                                        wap = w2[64 * b:64 * b + 64, ts(p, 64)]
                                        nc.tensor.ldweights(wap, tile_position=tp)
                                        nc.tensor.matmul(
                                            outap, wap,
                                            xt[64 * b:64 * b + 64,
                                               2 * j + dy:2 * j + dy + 2, dx:dx + W],
                                            start=(r == 0), stop=(r == 4 - par),
                                            tile_position=tp,
                                        )
                            # evacuate: stage[0:64]=block0, stage[64:128]=block1
                            dst = stage[:, ts(jj, 512)]
                            nc.scalar.activation(dst, psE[:, :],
                                                 mybir.ActivationFunctionType.Copy)
                            nc.vector.tensor_add(dst[0:64, :], dst[0:64, :],
                                                 psO[64:128, :])
                            nc.vector.tensor_add(dst[64:128, :], dst[64:128, :],
                                                 psO[0:64, :])
                        # flush: one DMA per block, 16 rows x 256 each
                        for b in range(2):
                            r0 = 64 * i + HB * b + 2 * FLUSH * half
                            nc.sync.dma_start(
                                out=out[:, r0:r0 + 2 * FLUSH, :],
                                in_=stage[64 * b:64 * b + 64, :],
                            )
    nc.finalize()
    return nc


_NC = None


def _get_nc():
    global _NC
    if _NC is None:
        _NC = build_nc()
    return _NC


def make_in_maps(x, s, weight):
    x = np.ascontiguousarray(np.asarray(x, dtype=np.float32))
    s = np.ascontiguousarray(np.asarray(s, dtype=np.float32))
    w = np.ascontiguousarray(np.asarray(weight, dtype=np.float32)).reshape(COUT, CIN * 9)
    return [
        {"x": x[c], "s": s[c:c + 1], "wgt": w}
        for c in range(B)
    ]


def run(x, s, weight, **kw):
    nc = _get_nc()
    res = run_bass_kernel_spmd(nc, make_in_maps(x, s, weight),
                               core_ids=list(range(B)), **kw)
    out = np.stack([r["out"] for r in res.results])  # [8, 64, 256, 256]
    return out, res


def kernel(x, s, weight):
    out, _ = run(x, s, weight)
    return out.astype(np.float32)


if __name__ == "__main__":
    rng = np.random.default_rng(0)
    xv = rng.standard_normal((B, CIN, H, W), dtype=np.float32)
    sv = rng.standard_normal((B, CIN), dtype=np.float32)
    wv = (rng.standard_normal((COUT, CIN, KK, KK), dtype=np.float32)
          * np.float32(np.sqrt(2.0 / (CIN * KK * KK))))
    o = kernel(xv, sv, wv)
    print("ran ok", o.shape, o.dtype, float(np.abs(o).max()))



# revision 6
# speedup vs baseline: 1.2791x; 1.2791x over previous
"""Trainium2 Bass kernel for nn_Conv2DMod (StyleGAN2-style modulated 3x3 conv).

Problem: x[8,64,256,256], s[8,64], weight[64,64,3,3] (f32)
  w = weight * (s+1) per sample; demod by rsqrt(sum w^2 over (Cin,K,K));
  out[b] = conv2d(x[b], w_b, pad=1).

Sharding: data-parallel over batch. 8 samples -> 8 NeuronCores, one each.

Per-core algorithm (V3):
  - host pre-pads x to [64,258,258] bf16 (zero halo rows+cols), pre-transposes
    weight to lhsT layout [i, p, o] replicated to 128 partitions, s as column.
  - weight prep on-chip, no PE transposes: modulate along partition dim (Cin),
    demod norm via ones-matmul row reduction + small adds, partition-broadcast
    of 1/sqrt, one fused scale into bf16 w2. Overlaps the first x DMA.
  - conv as shift-matmul over 9 kernel positions, 4 independent 64x64 PE cells
    (row tiles = block0/block1 x data, col tiles = even/odd kernel positions,
    crossed psum banks). Groups of 2 chunks (2x2 output rows, N=512 each)
    share each LDWEIGHTS; position 8 alternates col groups per chunk so every
    cell runs exactly 9 matmuls per group.
  - psum group tiles span 2 banks (chunk 0 -> bank A, chunk 1 -> bank B);
    2 bufs x (E,O) = all 8 banks.
  - evacuation per group, split across engines: ACT copies psE->stage (bf16)
    and psO[0:64]->tmp (bf16); DVE adds psO[64:128] into stage[0:64];
    GpSimd adds tmp into stage[64:128]. Output DMA'd as bf16 (host upcasts);
    x loads on HWDGE (sync), stores on SWDGE (gpsimd).
"""

import ml_dtypes
import numpy as np

import concourse.bacc as bacc
import concourse.mybir as mybir
import concourse.tile as tile
from concourse.bass import ts
from concourse.bass_utils import run_bass_kernel_spmd

F32 = mybir.dt.float32
BF16 = mybir.dt.bfloat16

B, CIN, COUT, KK, H, W = 8, 64, 64, 3, 256, 256
EPS = 1e-8
PW = W + 2          # padded row width
PH = H + 2          # padded height
HB = 32             # output rows per block
NBI = H // (2 * HB)  # pair-iterations (4)
NGRP = HB // 4      # 2-chunk groups per iteration (8)
FLUSH_G = 2         # groups per stage flush (4 chunks = 8 rows per block)

EVEN = [0, 2, 4, 6]
ODD = [1, 3, 5, 7]


def build_nc():
    nc = bacc.Bacc("TRN2")
    xp = nc.dram_tensor("xp", [CIN, PH, PW], BF16, kind="ExternalInput")
    sT = nc.dram_tensor("sT", [128, 1], F32, kind="ExternalInput")
    wgtT = nc.dram_tensor("wgtT", [128, 9 * 64], F32, kind="ExternalInput")
    out = nc.dram_tensor("out", [COUT, H, W], BF16, kind="ExternalOutput")

    with tile.TileContext(nc) as tc:
        with tc.tile_pool(name="const", bufs=1) as constp:
            w2 = constp.tile([128, 9, 64], BF16)   # [i, p, o] lhsT per position

            # ---- weight prep (f32 math, bf16 result), no transposes ----
            with (
                tc.tile_pool(name="prep", bufs=1) as prepp,
                tc.tile_pool(name="prep_ps", bufs=1, space="PSUM") as prep_ps,
            ):
                wT = prepp.tile([128, 9, 64], F32)    # [i, p, o]
                nc.sync.dma_start(out=wT[:, :, :], in_=wgtT[:, :])
                sp1 = prepp.tile([128, 1], F32)
                nc.sync.dma_start(out=sp1[:, :], in_=sT[:, :])
                nc.vector.tensor_scalar_add(sp1[:, :], sp1[:, :], 1.0)
                # modulate: wmod[i,p,o] = wT * (s[i]+1)
                wmod = prepp.tile([128, 9, 64], F32)
                nc.vector.tensor_scalar_mul(wmod[:, :, :], wT[:, :, :], sp1[:, :])
                # demod norm: d2[o] = sum_i sum_p wmod^2  (rows 0:64 suffice)
                sq = prepp.tile([64, 9, 64], F32)
                nc.vector.tensor_mul(sq[:, :, :], wmod[0:64, :, :], wmod[0:64, :, :])
                ones = prepp.tile([64, 1], F32)
                nc.vector.memset(ones[:, :], 1.0)
                psA = prep_ps.tile([64, 512], F32)
                psB = prep_ps.tile([64, 64], F32)
                nc.tensor.matmul(psA[0:1, :], ones[:, 0:1], sq[:, 0:8, :],
                                 start=True, stop=True)
                nc.tensor.matmul(psB[0:1, :], ones[:, 0:1], sq[:, 8, :],
                                 start=True, stop=True)
                srow = prepp.tile([1, 576], F32)
                nc.vector.tensor_copy(srow[0:1, 0:512], psA[0:1, :])
                nc.vector.tensor_copy(srow[0:1, 512:576], psB[0:1, :])
                acc = prepp.tile([1, 64], F32)
                nc.vector.tensor_add(acc[0:1, :], srow[0:1, 0:64],
                                     srow[0:1, 64:128])
                for k in range(2, 9):
                    nc.vector.tensor_add(acc[0:1, :], acc[0:1, :],
                                         srow[0:1, ts(k, 64)])
                epst = prepp.tile([1, 1], F32)
                nc.vector.memset(epst[:, :], EPS)
                dtmp = prepp.tile([1, 64], F32)
                nc.scalar.activation(dtmp[0:1, :], acc[0:1, :],
                                     mybir.ActivationFunctionType.Sqrt,
                                     bias=epst[0:1, 0:1])
                dinv = prepp.tile([1, 64], F32)
                nc.vector.reciprocal(dinv[0:1, :], dtmp[0:1, :])
                # broadcast d across partitions via K=1 outer product
                onesr = prepp.tile([1, 128], F32)
                nc.vector.memset(onesr[:, :], 1.0)
                psD = prep_ps.tile([128, 64], F32)
                nc.tensor.matmul(psD[:, :], onesr[0:1, :], dinv[0:1, :],
                                 start=True, stop=True)
                # final: w2[i,p,o] = wmod * d[o]  (bf16)
                nc.vector.tensor_mul(
                    w2[:, :, :], wmod[:, :, :],
                    psD[:, :].unsqueeze(1).to_broadcast((128, 9, 64)),
                )

            # ---- main conv loop ----
            with (
                tc.tile_pool(name="xpool", bufs=2) as xpool,
                tc.tile_pool(name="stpool", bufs=2) as stpool,
                tc.tile_pool(name="tmpool", bufs=2) as tmpool,
                tc.tile_pool(name="pspool", bufs=2, space="PSUM") as pspool,
            ):
                for i in range(NBI):
                    xt = xpool.tile([128, HB + 2, PW], BF16, name=f"xt{i}",
                                    tag="xt")
                    lo0 = 64 * i          # block0 padded rows [lo0, lo0+34)
                    lo1 = 64 * i + HB     # block1 padded rows [lo1, lo1+34)
                    if i == 0:
                        # split so the first groups' rows land fast
                        nc.sync.dma_start(out=xt[0:64, 0:12, :],
                                          in_=xp[:, 0:12, :])
                        nc.sync.dma_start(out=xt[64:128, 0:12, :],
                                          in_=xp[:, lo1:lo1 + 12, :])
                        nc.sync.dma_start(out=xt[0:64, 12:HB + 2, :],
                                          in_=xp[:, 12:HB + 2, :])
                        nc.sync.dma_start(out=xt[64:128, 12:HB + 2, :],
                                          in_=xp[:, lo1 + 12:lo1 + HB + 2, :])
                    else:
                        nc.sync.dma_start(out=xt[0:64, :, :],
                                          in_=xp[:, lo0:lo0 + HB + 2, :])
                        nc.sync.dma_start(out=xt[64:128, :, :],
                                          in_=xp[:, lo1:lo1 + HB + 2, :])

                    for gg in range(NGRP // FLUSH_G):
                        stage = stpool.tile([128, FLUSH_G * 1024], BF16,
                                            name=f"stage{i}_{gg}", tag="stage")
                        for gj in range(FLUSH_G):
                            g = gg * FLUSH_G + gj
                            psE = pspool.tile([128, 1024], F32,
                                              name=f"psE{i}_{g}", tag="psE")
                            psO = pspool.tile([128, 1024], F32,
                                              name=f"psO{i}_{g}", tag="psO")
                            # cells: (b, col h0)=even pos, (b, col h64)=odd pos
                            #  b0 even->psE[0:64], b1 even->psO[0:64]
                            #  b0 odd ->psO[64:128], b1 odd->psE[64:128]
                            for k in range(5):
                                for par in range(2):   # 0=col h0, 1=col h64
                                    if k == 4:
                                        p = 8
                                        jjs = [par]    # p=8: jj0 on h0, jj1 on h64
                                    else:
                                        p = (EVEN, ODD)[par][k]
                                        jjs = [0, 1]
                                    dy, dx = divmod(p, 3)
                                    for b in range(2):
                                        if par == 0:
                                            ps = (psE, psO)[b]
                                            pr = slice(0, 64)
                                            tp = (64 * b, 0)
                                        else:
                                            ps = (psO, psE)[b]
                                            pr = slice(64, 128)
                                            tp = (64 * b, 64)
                                        wap = w2[64 * b:64 * b + 64, p, :]
                                        nc.tensor.ldweights(wap, tile_position=tp)
                                        for jj in jjs:
                                            c = 2 * g + jj
                                            # start: first MM of this (cell,chunk)
                                            # evens: jj0 k0..k4(5), jj1 k0..k3(4)
                                            # odds:  jj0 k0..k3(4), jj1 k0..k4(5)
                                            st = (k == 0)
                                            if par == 0:
                                                sp = (k == 4) if jj == 0 else (k == 3)
                                            else:
                                                sp = (k == 3) if jj == 0 else (k == 4)
                                            nc.tensor.matmul(
                                                ps[pr, ts(jj, 512)], wap,
                                                xt[64 * b:64 * b + 64,
                                                   2 * c + dy:2 * c + dy + 2,
                                                   dx:dx + W],
                                                start=st, stop=sp,
                                                tile_position=tp,
                                            )
                            # evacuate group: stage rows [4*gj .. 4*gj+4)
                            dst = stage[:, ts(gj, 1024)]
                            tmp = tmpool.tile([128, 1024], BF16,
                                              name=f"tmp{i}_{g}", tag="tmp")
                            nc.scalar.activation(dst, psE[:, :],
                                                 mybir.ActivationFunctionType.Copy)
                            nc.scalar.activation(tmp[64:128, :], psO[0:64, :],
                                                 mybir.ActivationFunctionType.Copy)
                            nc.vector.tensor_add(dst[0:64, :], dst[0:64, :],
                                                 psO[64:128, :])
                            nc.gpsimd.tensor_add(dst[64:128, :], dst[64:128, :],
                                                 tmp[64:128, :])
                        # flush: one DMA per block, 8 rows x 256 each
                        for b in range(2):
                            r0 = 64 * i + HB * b + 4 * FLUSH_G * gg
                            nc.gpsimd.dma_start(
                                out=out[:, r0:r0 + 4 * FLUSH_G, :],
                                in_=stage[64 * b:64 * b + 64, :],
                            )
    nc.finalize()
    return nc


_NC = None


def _get_nc():
    global _NC
    if _NC is None:
        _NC = build_nc()
    return _NC


def make_in_maps(x, s, weight):
    x = np.asarray(x, dtype=np.float32)
    s = np.asarray(s, dtype=np.float32)
    w = np.asarray(weight, dtype=np.float32)
    wT = w.reshape(COUT, CIN, 9).transpose(1, 2, 0).reshape(CIN, 9 * COUT)
    wT2 = np.ascontiguousarray(np.concatenate([wT, wT], axis=0))  # [128, 576]
    xpad = np.zeros((B, CIN, PH, PW), dtype=ml_dtypes.bfloat16)
    xpad[:, :, 1:H + 1, 1:W + 1] = x
    maps = []
    for c in range(B):
        sT2 = np.ascontiguousarray(
            np.tile(s[c][:, None], (2, 1)).astype(np.float32))  # [128, 1]
        maps.append({"xp": xpad[c], "sT": sT2, "wgtT": wT2})
    return maps


def run(x, s, weight, **kw):
    nc = _get_nc()
    res = run_bass_kernel_spmd(nc, make_in_maps(x, s, weight),
                               core_ids=list(range(B)), **kw)
    out = np.stack([np.asarray(r["out"]) for r in res.results])
    return out, res


def kernel(x, s, weight):
    out, _ = run(x, s, weight)
    return out.astype(np.float32)


if __name__ == "__main__":
    rng = np.random.default_rng(0)
    xv = rng.standard_normal((B, CIN, H, W), dtype=np.float32)
    sv = rng.standard_normal((B, CIN), dtype=np.float32)
    wv = (rng.standard_normal((COUT, CIN, KK, KK), dtype=np.float32)
          * np.float32(np.sqrt(2.0 / (CIN * KK * KK))))
    o = kernel(xv, sv, wv)
    print("ran ok", o.shape, o.dtype, float(np.abs(o).max()))


# revision 10
# speedup vs baseline: 1.4137x; 1.1052x over previous
"""Trainium2 Bass kernel for nn_Conv2DMod (StyleGAN2-style modulated 3x3 conv).

Problem: x[8,64,256,256], s[8,64], weight[64,64,3,3] (f32)
  w = weight * (s+1) per sample; demod by rsqrt(sum w^2 over (Cin,K,K));
  out[b] = conv2d(x[b], w_b, pad=1).

Sharding: data-parallel over batch. 8 samples -> 8 NeuronCores, one each.

Per-core algorithm (V5):
  - host pre-pads x to [64,258,258] bf16 (zero halo rows+cols), pre-transposes
    weight to lhsT layout [i, p, o] replicated to 128 partitions, s as column.
  - all 4 x row-slab loads are issued first on the sync (HWDGE) queue
    (xpool bufs=4, so none waits on buffer recycling); weight/s prep inputs go
    on the scalar queue so the two never serialize.
  - conv runs with MODULATED but UNdemodulated weights (w2 = wT*(s+1), bf16),
    ready as soon as the weight DMA + one DVE pass finish. The demod scale
    d[o] = rsqrt(sum w^2) is computed concurrently as a [128,1] column
    (duplicated halves, via two K=1 transpose matmuls) and applied during
    evacuation as a per-partition scale.
  - conv as shift-matmul over 9 kernel positions, 4 independent 64x64 PE cells
    (row tiles = block0/block1 x data, col tiles = even/odd kernel positions,
    crossed psum banks). Groups of 2 chunks (2x2 output rows, N=512 each)
    share each LDWEIGHTS; position 8 alternates col groups per chunk so every
    cell runs exactly 9 matmuls per group.
  - psum group tiles span 2 banks; 2 bufs x (E,O) = all 8 banks.
  - evacuation per group, engines balanced under the PE's ~2.2us/group:
      DVE: stageB = psE * d2   (tensor_scalar 2x rate from PSUM, frees psE)
      ACT: stage[0:64]  = psO[64:128] * d2 (cross-base copy w/ scale)
      ACT: stage[64:128] = psO[0:64]  * d2 (frees psO)
      DVE: stage += stageB     (aligned full-width bf16 add, 2x rate)
    Output DMA'd as bf16 (host upcasts); stores on SWDGE (gpsimd queue).
"""

import ml_dtypes
import numpy as np

import concourse.bacc as bacc
import concourse.mybir as mybir
import concourse.tile as tile
from concourse.bass import ts
from concourse.bass_utils import run_bass_kernel_spmd

F32 = mybir.dt.float32
BF16 = mybir.dt.bfloat16

B, CIN, COUT, KK, H, W = 8, 64, 64, 3, 256, 256
EPS = 1e-8
PW = W + 2          # padded row width
PH = H + 2          # padded height
HB = 32             # output rows per block
NBI = H // (2 * HB)  # pair-iterations (4)
NGRP = HB // 4      # 2-chunk groups per iteration (8)
FLUSH_G = 2         # groups per stage flush (4 chunks = 8 rows per block)

EVEN = [0, 2, 4, 6]
ODD = [1, 3, 5, 7]


def emit_x_load(nc, xt, xp, i):
    lo0 = 64 * i          # block0 padded rows [lo0, lo0+34)
    lo1 = 64 * i + HB     # block1 padded rows [lo1, lo1+34)
    if i == 0:
        # split so the first groups' rows land fast
        nc.sync.dma_start(out=xt[0:64, 0:12, :], in_=xp[:, 0:12, :])
        nc.sync.dma_start(out=xt[64:128, 0:12, :], in_=xp[:, lo1:lo1 + 12, :])
        nc.sync.dma_start(out=xt[0:64, 12:HB + 2, :], in_=xp[:, 12:HB + 2, :])
        nc.sync.dma_start(out=xt[64:128, 12:HB + 2, :],
                          in_=xp[:, lo1 + 12:lo1 + HB + 2, :])
    else:
        nc.sync.dma_start(out=xt[0:64, :, :], in_=xp[:, lo0:lo0 + HB + 2, :])
        nc.sync.dma_start(out=xt[64:128, :, :], in_=xp[:, lo1:lo1 + HB + 2, :])


def build_nc():
    nc = bacc.Bacc("TRN2")
    xp = nc.dram_tensor("xp", [CIN, PH, PW], BF16, kind="ExternalInput")
    sT = nc.dram_tensor("sT", [128, 1], F32, kind="ExternalInput")
    wgtT = nc.dram_tensor("wgtT", [128, 9 * 64], F32, kind="ExternalInput")
    out = nc.dram_tensor("out", [COUT, H, W], BF16, kind="ExternalOutput")

    with (
        tile.TileContext(nc) as tc,
        tc.tile_pool(name="const", bufs=1) as constp,
        tc.tile_pool(name="xpool", bufs=NBI) as xpool,
    ):
        w2 = constp.tile([128, 9, 64], BF16)   # [i, p, o] modulated lhsT
        d2 = constp.tile([128, 1], F32)        # demod scale column (dup halves)

        # all x slab loads first in program order -> sync HWDGE queue streams
        # them back to back from t=0
        xts = []
        for i in range(NBI):
            xts.append(xpool.tile([128, HB + 2, PW], BF16, name=f"xt{i}",
                                  tag="xt"))
            emit_x_load(nc, xts[i], xp, i)

        # ---- weight prep (f32 math, bf16 result), no transposes ----
        with (
            tc.tile_pool(name="prep", bufs=1) as prepp,
            tc.tile_pool(name="prep_ps", bufs=1, space="PSUM") as prep_ps,
        ):
            wT = prepp.tile([128, 9, 64], F32)    # [i, p, o]
            nc.scalar.dma_start(out=wT[:, :, :], in_=wgtT[:, :])
            sp1 = prepp.tile([128, 1], F32)
            nc.scalar.dma_start(out=sp1[:, :], in_=sT[:, :])
            nc.vector.tensor_scalar_add(sp1[:, :], sp1[:, :], 1.0)
            # modulate straight into bf16 conv weights (demod applied at evac)
            nc.vector.tensor_scalar_mul(w2[:, :, :], wT[:, :, :], sp1[:, :])
            # demod norm: d2[o] = rsqrt(sum_i sum_p wmod^2 + eps) as a column
            wmod = prepp.tile([64, 9, 64], F32)
            nc.vector.tensor_scalar_mul(wmod[:, :, :], wT[0:64, :, :],
                                        sp1[0:64, :])
            sq = prepp.tile([64, 9, 64], F32)
            nc.vector.tensor_mul(sq[:, :, :], wmod[:, :, :], wmod[:, :, :])
            ones = prepp.tile([64, 1], F32)
            nc.vector.memset(ones[:, :], 1.0)
            psA = prep_ps.tile([64, 512], F32)
            psB = prep_ps.tile([64, 64], F32)
            nc.tensor.matmul(psA[0:1, :], ones[:, 0:1], sq[:, 0:8, :],
                             start=True, stop=True)
            nc.tensor.matmul(psB[0:1, :], ones[:, 0:1], sq[:, 8, :],
                             start=True, stop=True)
            acc = prepp.tile([1, 64], F32)
            nc.vector.tensor_copy(acc[0:1, :], psA[0:1, 0:64])
            for k in range(1, 8):
                nc.vector.tensor_add(acc[0:1, :], acc[0:1, :],
                                     psA[0:1, ts(k, 64)])
            nc.vector.tensor_add(acc[0:1, :], acc[0:1, :], psB[0:1, :])
            epst = prepp.tile([1, 1], F32)
            nc.vector.memset(epst[:, :], EPS)
            dtmp = prepp.tile([1, 64], F32)
            nc.scalar.activation(dtmp[0:1, :], acc[0:1, :],
                                 mybir.ActivationFunctionType.Sqrt,
                                 bias=epst[0:1, 0:1])
            dinv = prepp.tile([1, 64], F32)
            nc.vector.reciprocal(dinv[0:1, :], dtmp[0:1, :])
            # row -> column (both halves) via K=1 transpose matmuls
            ones1 = prepp.tile([1, 1], F32)
            nc.vector.memset(ones1[:, :], 1.0)
            psD = prep_ps.tile([128, 1], F32)
            nc.tensor.matmul(psD[0:64, 0:1], dinv[0:1, :], ones1[0:1, :],
                             start=True, stop=True)
            nc.tensor.matmul(psD[64:128, 0:1], dinv[0:1, :], ones1[0:1, :],
                             start=True, stop=True, tile_position=(0, 64))
            nc.vector.tensor_copy(d2[:, :], psD[:, :])

        # ---- main conv loop ----
        with (
            tc.tile_pool(name="stpool", bufs=2) as stpool,
            tc.tile_pool(name="sbpool", bufs=2) as sbpool,
            tc.tile_pool(name="pspool", bufs=2, space="PSUM") as pspool,
        ):
            for i in range(NBI):
                xt = xts[i]
                for gg in range(NGRP // FLUSH_G):
                    stage = stpool.tile([128, FLUSH_G * 1024], BF16,
                                        name=f"stage{i}_{gg}", tag="stage")
                    for gj in range(FLUSH_G):
                        g = gg * FLUSH_G + gj
                        psE = pspool.tile([128, 1024], F32,
                                          name=f"psE{i}_{g}", tag="psE")
                        psO = pspool.tile([128, 1024], F32,
                                          name=f"psO{i}_{g}", tag="psO")
                        # cells: (b, col h0)=even pos, (b, col h64)=odd pos
                        #  b0 even->psE[0:64], b1 even->psO[0:64]
                        #  b0 odd ->psO[64:128], b1 odd->psE[64:128]
                        for k in range(5):
                            for par in range(2):   # 0=col h0, 1=col h64
                                if k == 4:
                                    p = 8
                                    jjs = [par]    # p=8: jj0 on h0, jj1 on h64
                                else:
                                    p = (EVEN, ODD)[par][k]
                                    jjs = [0, 1]
                                dy, dx = divmod(p, 3)
                                for b in range(2):
                                    if par == 0:
                                        ps = (psE, psO)[b]
                                        pr = slice(0, 64)
                                        tp = (64 * b, 0)
                                    else:
                                        ps = (psO, psE)[b]
                                        pr = slice(64, 128)
                                        tp = (64 * b, 64)
                                    wap = w2[64 * b:64 * b + 64, p, :]
                                    nc.tensor.ldweights(wap, tile_position=tp)
                                    for jj in jjs:
                                        c = 2 * g + jj
                                        # evens: jj0 k0..4 (5), jj1 k0..3 (4)
                                        # odds:  jj0 k0..3 (4), jj1 k0..4 (5)
                                        st = (k == 0)
                                        if par == 0:
                                            sp = (k == 4) if jj == 0 else (k == 3)
                                        else:
                                            sp = (k == 3) if jj == 0 else (k == 4)
                                        nc.tensor.matmul(
                                            ps[pr, ts(jj, 512)], wap,
                                            xt[64 * b:64 * b + 64,
                                               2 * c + dy:2 * c + dy + 2,
                                               dx:dx + W],
                                            start=st, stop=sp,
                                            tile_position=tp,
                                        )
                        # evacuate group: stage rows [4*gj .. 4*gj+4)
                        dst = stage[:, ts(gj, 1024)]
                        stageB = sbpool.tile([128, 1024], BF16,
                                             name=f"stgB{i}_{g}", tag="stgB")
                        nc.vector.tensor_scalar_mul(stageB[:, :], psE[:, :],
                                                    d2[:, :])
                        nc.scalar.activation(dst[0:64, :], psO[64:128, :],
                                             mybir.ActivationFunctionType.Copy,
                                             scale=d2[64:128, :])
                        nc.scalar.activation(dst[64:128, :], psO[0:64, :],
                                             mybir.ActivationFunctionType.Copy,
                                             scale=d2[0:64, :])
                        nc.vector.tensor_add(dst, dst, stageB[:, :])
                    # flush: one DMA per block, 8 rows x 256 each
                    for b in range(2):
                        r0 = 64 * i + HB * b + 4 * FLUSH_G * gg
                        nc.gpsimd.dma_start(
                            out=out[:, r0:r0 + 4 * FLUSH_G, :],
                            in_=stage[64 * b:64 * b + 64, :],
                        )
    nc.finalize()
    return nc


_NC = None


def _get_nc():
    global _NC
    if _NC is None:
        _NC = build_nc()
    return _NC


def make_in_maps(x, s, weight):
    x = np.asarray(x, dtype=np.float32)
    s = np.asarray(s, dtype=np.float32)
    w = np.asarray(weight, dtype=np.float32)
    wT = w.reshape(COUT, CIN, 9).transpose(1, 2, 0).reshape(CIN, 9 * COUT)
    wT2 = np.ascontiguousarray(np.concatenate([wT, wT], axis=0))  # [128, 576]
    xpad = np.zeros((B, CIN, PH, PW), dtype=ml_dtypes.bfloat16)
    xpad[:, :, 1:H + 1, 1:W + 1] = x
    maps = []
    for c in range(B):
        sT2 = np.ascontiguousarray(
            np.tile(s[c][:, None], (2, 1)).astype(np.float32))  # [128, 1]
        maps.append({"xp": xpad[c], "sT": sT2, "wgtT": wT2})
    return maps


def run(x, s, weight, **kw):
    nc = _get_nc()
    res = run_bass_kernel_spmd(nc, make_in_maps(x, s, weight),
                               core_ids=list(range(B)), **kw)
    out = np.stack([np.asarray(r["out"]) for r in res.results])
    return out, res


def kernel(x, s, weight):
    out, _ = run(x, s, weight)
    return out.astype(np.float32)


if __name__ == "__main__":
    rng = np.random.default_rng(0)
    xv = rng.standard_normal((B, CIN, H, W), dtype=np.float32)
    sv = rng.standard_normal((B, CIN), dtype=np.float32)
    wv = (rng.standard_normal((COUT, CIN, KK, KK), dtype=np.float32)
          * np.float32(np.sqrt(2.0 / (CIN * KK * KK))))
    o = kernel(xv, sv, wv)
    print("ran ok", o.shape, o.dtype, float(np.abs(o).max()))
